# revision 64
# baseline (speedup 1.0000x reference)
"""Causal linear attention (elu+1 feature map) for Trainium2, 8 NeuronCores.

Problem: B=2, S=2048, D=1024, H=16, HD=64.
  q/k/v projections [S,D]@[D,H*HD], phi = elu+1, causal linear attention
  out[t] = (sum_{i<=t} (phi_q[t].phi_k[i]) v[i]) / (phi_q[t].sum_{i<=t} phi_k[i] + eps)

Sharding: core c -> (batch b=c//4, heads h0=4*(c%4) .. h0+3). No cross-core comm.
Host feeds x^T [D,S] per core in bf16 (halves HBM traffic; harness rel-err
budget is 2e-2, bf16-in costs ~2e-3). Every matmul streams a bf16 moving
operand (full PE rate incl. small attention matmuls); PSUM stays fp32.

PSUM semantics (validated on HW): matmul start=True lazily marks the whole
2KB bank pending-zero; each byte's first write after that overwrites, later
writes accumulate. Within one psum tile, matmuls that change the stationary
base partition across different output regions wedge the device (lowering
bug) - so every multi-region tile here keeps a uniform base partition, and
mixed-base pairs (the per-head qS) always target the same region.

Device algorithm (per core, 4 heads, chunked L=128):
  - proj q,k -> phi_qT/phi_kT bf16, head-pairs packed on 128 partitions
  - phi_k seq-major via PE transposes (2 per chunk into one psum tile)
  - v seq-major with appended ones column (v_aug [*,65] per head); the ones
    column makes col 64 of each o-block the normalizer qz
  - per chunk: A for the two even heads into one psum bank (base 0) and the
    two odd heads into another (base 64); two DVE mask-mults -> per-head
    psum o-tile accumulates tril(A)@v_aug + q@S_prev (bf16 state shadow);
    KV state accumulates in a persistent psum bank, snapshotted per chunk
  - normalize: per-head DVE reciprocal + Act copy-with-scale multiply
"""

import threading

import numpy as np

B, S, D, H, HD = 2, 2048, 1024, 16, 64
EPS = 1e-6
N_CORES = 8
HPC = 4            # heads per core
HDC = HPC * HD     # 256 projected cols per core
NCHUNK = S // 128  # 16
DC = D // 128      # 8 contraction chunks

_lock = threading.Lock()
_cache = {}

FLAGS: set = set()  # debug: "f32out" = fp32 output tensor


def _build_nc():
    import concourse.tile as tile
    from concourse import bacc, mybir

    f32 = mybir.dt.float32
    bf16 = mybir.dt.bfloat16
    Alu = mybir.AluOpType
    Act = mybir.ActivationFunctionType

    nc = bacc.Bacc("TRN2", target_bir_lowering=False, debug=False)

    xqT = nc.dram_tensor("xqT", [D, S], bf16, kind="ExternalInput").ap()
    xkT = nc.dram_tensor("xkT", [D, S], bf16, kind="ExternalInput").ap()
    xvT = nc.dram_tensor("xvT", [D, S], bf16, kind="ExternalInput").ap()
    wq = nc.dram_tensor("wq", [D, HDC], bf16, kind="ExternalInput").ap()
    wk = nc.dram_tensor("wk", [D, HDC], bf16, kind="ExternalInput").ap()
    wv = nc.dram_tensor("wv", [D, HDC], bf16, kind="ExternalInput").ap()
    odt = f32 if "f32out" in FLAGS else bf16
    out = nc.dram_tensor("out", [S, HDC], odt, kind="ExternalOutput").ap()

    with tile.TileContext(nc) as tc:
        with (
            tc.tile_pool(name="consts", bufs=1) as consts,
            tc.tile_pool(name="weights", bufs=1) as wpool,
            tc.tile_pool(name="resident", bufs=1) as res,
            tc.tile_pool(name="xin", bufs=48) as xin,
            tc.tile_pool(name="work", bufs=3) as work,
            tc.tile_pool(name="attn", bufs=3) as attn,
            tc.tile_pool(name="psum", bufs=2, space="PSUM") as psum,
            tc.tile_pool(name="spsum", bufs=1, space="PSUM") as spool,
        ):
            # ---- constants ----
            onesb = consts.tile([128, 128], bf16)
            nc.vector.memset(onesb[:], 1.0)
            identb = consts.tile([128, 128], bf16)
            nc.gpsimd.affine_select(
                identb[:], onesb[:], pattern=[[-1, 128]], base=0,
                channel_multiplier=1, compare_op=Alu.is_equal, fill=0.0,
            )
            ones2 = consts.tile([128, 256], f32)
            nc.vector.memset(ones2[:], 1.0)
            # causal mask, [j (part), t (free)] layout, tiled 2x: keep j <= t
            mask2 = consts.tile([128, 256], f32)
            nc.gpsimd.affine_select(
                mask2.rearrange("p (h t) -> p h t", h=2)[:],
                ones2.rearrange("p (h t) -> p h t", h=2)[:],
                pattern=[[0, 2], [1, 128]], base=0,
                channel_multiplier=-1, compare_op=Alu.is_ge, fill=0.0,
            )

            # ---- weights + x^T loads, ordered so the first projection can
            # start ~3us in: wq, then quarter-0 q pieces, wk, k pieces, ...
            w_sb = {}
            xt = {}

            def load_w(name, wdram):
                # two halves so the first projection matmul (needs dc 0 only)
                # starts ~0.7us after queue head instead of 1.5us
                wt = wpool.tile([128, DC, HDC], bf16, name=f"w{name}_sb")
                src = wdram.rearrange("(dc p) m -> p dc m", p=128)
                nc.sync.dma_start(wt[:, 0:4, :], src[:, 0:4, :])
                nc.sync.dma_start(wt[:, 4:8, :], src[:, 4:8, :])
                w_sb[name] = wt

            def load_x(tname, xdram, qt):
                pieces = []
                for ph in range(4):
                    t = xin.tile([128, 2, 512], bf16,
                                 name=f"x_{tname}_{qt}_{ph}", tag="xin")
                    src = xdram.rearrange("(dc p) s -> p dc s", p=128)
                    nc.sync.dma_start(
                        t[:], src[:, ph * 2:(ph + 1) * 2,
                                  qt * 512:(qt + 1) * 512])
                    pieces.append(t)
                xt[(tname, qt)] = pieces

            for name, wdram, xdram in (("k", wk, xkT), ("q", wq, xqT),
                                       ("v", wv, xvT)):
                load_w(name, wdram)
                load_x(name, xdram, 0)
            for qt in range(1, 4):
                for tname, xdram in (("k", xkT), ("q", xqT), ("v", xvT)):
                    load_x(tname, xdram, qt)

            # ---- resident activations ----
            phi_qT = [res.tile([128, S], bf16, name=f"phi_qT{i}") for i in range(2)]
            phi_kT = [res.tile([128, S], bf16, name=f"phi_kT{i}") for i in range(2)]
            phi_ks = res.tile([128, NCHUNK * HDC], bf16, name="phi_ks")
            v_aug = res.tile([128, NCHUNK * HPC * 65], bf16, name="v_aug")
            vaug4 = v_aug.rearrange("p (c h e) -> p c h e", c=NCHUNK, h=HPC)
            nc.vector.memset(vaug4[:, :, :, 64:65], 1.0)

            # persistent KV-state accumulator (psum fp32, whole kernel); per
            # pair hp: even head parts 0..63 cols hp*130..+64, odd head parts
            # 64..127 cols hp*130+65..+129. Started (lazy bank zero) exactly
            # once; stop is sim-only and closes the group each chunk so the
            # snapshot reads stay legal while HW keeps accumulating.
            S_ps = spool.tile([128, 2 * 130], f32, name="S_ps")

            def phi_from_psum(ps, dst):
                # phi(x) = exp(min(x,0)) + max(x,0); m' = relu(-x); e = exp(-m')
                t1 = work.tile([128, 512], f32, tag="phi1", bufs=4)
                nc.scalar.activation(t1[:], ps[:], Act.Relu, scale=-1.0)
                t2 = work.tile([128, 512], f32, tag="phi2", bufs=4)
                nc.scalar.activation(t2[:], t1[:], Act.Exp, scale=-1.0)
                nc.vector.scalar_tensor_tensor(
                    dst, ps[:], 0.0, t2[:], op0=Alu.max, op1=Alu.add)

            S_sb_prev = None
            a_pss = {}
            for qt in range(4):
                s0 = qt * 512
                # q/k projections (hd-major, head pair per psum bank)
                for tname, dst in (("k", phi_kT), ("q", phi_qT)):
                    xh = xt[(tname, qt)]
                    for hp in range(2):
                        ps = psum.tile([128, 512], f32, tag="proj", bufs=3,
                                       name=f"ps_{tname}_{qt}_{hp}")
                        for dc in range(DC):
                            nc.tensor.matmul(
                                ps[:], w_sb[tname][:, dc, hp * 128:(hp + 1) * 128],
                                xh[dc // 2][:, dc % 2, :],
                                start=(dc == 0), stop=(dc == DC - 1),
                            )
                        phi_from_psum(ps, dst[hp][:, s0:s0 + 512])

                # v projection, seq-major, two chunks per psum bank (the
                # first start lazily zeroes the bank; region writes then
                # overwrite-first, accumulate-after)
                xh = xt[("v", qt)]
                for half in range(2):
                    c0 = qt * 4 + half * 2
                    psv = psum.tile([128, 512], f32, tag="proj", bufs=3,
                                    name=f"ps_v_{c0}")
                    for cc2 in range(2):
                        for dc in range(DC):
                            nc.tensor.matmul(
                                psv[:, cc2 * 256:(cc2 + 1) * 256],
                                xh[dc // 2][:, dc % 2,
                                            (half * 2 + cc2) * 128:
                                            (half * 2 + cc2 + 1) * 128],
                                w_sb["v"][:, dc, :],
                                start=(cc2 == 0 and dc == 0),
                                stop=(cc2 == 1 and dc == DC - 1),
                                skip_group_check=True,
                            )
                    nc.vector.tensor_copy(
                        vaug4[:, c0:c0 + 2, :, 0:64],
                        psv.rearrange("p (c h e) -> p c h e", c=2, h=HPC)[:])

                # phi_k seq-major via PE transposes (2 per chunk, one psum
                # tile, base 0 throughout)
                for cc in range(4):
                    c = qt * 4 + cc
                    tp = psum.tile([128, 256], bf16, tag="proj", bufs=3,
                                   name=f"tp_{c}")
                    for hp in range(2):
                        nc.tensor.transpose(
                            tp[:, hp * 128:(hp + 1) * 128],
                            phi_kT[hp][:, c * 128:(c + 1) * 128], identb[:])
                    nc.scalar.copy(phi_ks[:, c * HDC:(c + 1) * HDC], tp[:])

                # attention. A matmuls for the two heads of a PARITY share a
                # psum tile (uniform stationary base: 0 for even heads, 64
                # for odd); chunk c+1's A is issued before chunk c's
                # psum-dependent work so the DVE mask latency stays off the
                # PE critical path.
                def a4_mm(c):
                    tiles = []
                    for par in range(2):
                        a_ps = psum.tile([128, 256], f32, tag="A", bufs=2,
                                         name=f"a_ps_{c}_{par}")
                        hb = 64 * par
                        for hp in range(2):
                            nc.tensor.matmul(
                                a_ps[:, hp * 128:(hp + 1) * 128],
                                phi_kT[hp][hb:hb + 64, c * 128:(c + 1) * 128],
                                phi_qT[hp][hb:hb + 64, c * 128:(c + 1) * 128],
                                start=(hp == 0), stop=(hp == 1),
                                skip_group_check=True,
                            )
                        tiles.append(a_ps)
                    a_pss[c] = tiles

                a4_mm(qt * 4)
                for cc in range(4):
                    c = qt * 4 + cc
                    if cc < 3:
                        a4_mm(c + 1)
                    # masked A (bf16): one DVE op per parity
                    a_sb = attn.tile([128, 2, 256], bf16, tag="Asb", bufs=4,
                                     name=f"a_sb_{c}")
                    ap_e, ap_o = a_pss.pop(c)
                    nc.vector.tensor_tensor(a_sb[:, 0, :], ap_e[:], mask2[:],
                                            op=Alu.mult)
                    nc.vector.tensor_tensor(a_sb[:, 1, :], ap_o[:], mask2[:],
                                            op=Alu.mult)
                    # state update (persistent, started once at c==0/hp==0)
                    for hp in range(2):
                        nc.tensor.matmul(
                            S_ps[:, hp * 130:(hp + 1) * 130],
                            phi_ks[:, c * HDC + hp * 128:
                                   c * HDC + (hp + 1) * 128],
                            vaug4[:, c, 2 * hp:2 * hp + 2, :],
                            start=(c == 0 and hp == 0),
                            stop=(hp == 1),
                            skip_group_check=True,
                        )
                    # per-head o tiles: tril(A)@v_aug then q@S_prev (mixed
                    # stationary bases but same region - baseline-proven)
                    o_pss = {}
                    for h in range(HPC):
                        hp, par = h // 2, h % 2
                        op1 = psum.tile([128, 65], f32, tag="o", bufs=2,
                                        name=f"op1_{c}_{h}")
                        nc.tensor.matmul(
                            op1[:],
                            a_sb[:, par, hp * 128:(hp + 1) * 128],
                            vaug4[:, c, h, :],
                            start=True, stop=(c == 0))
                        if c > 0:
                            nc.tensor.matmul(
                                op1[:],
                                phi_qT[hp][64 * par:64 * par + 64,
                                           c * 128:(c + 1) * 128],
                                S_sb_prev[64 * par:64 * par + 64,
                                          hp * 130 + par * 65:
                                          hp * 130 + par * 65 + 65],
                                start=False, stop=True)
                        o_pss[h] = op1
                    # snapshot state for the next chunk's qS matmuls
                    S_sb = attn.tile([128, 2 * 130], bf16, tag="Ssb", bufs=3,
                                     name=f"S_sb_{c}")
                    nc.scalar.copy(S_sb[:], S_ps[:])
                    S_sb_prev = S_sb
                    # normalize: qz (col 64) is a sum of >=64 strictly
                    # positive terms, so the reference's +1e-6 folds away.
                    # reciprocal on DVE, multiply on Act (copy with scale AP)
                    if cc == 0:
                        o_sbq = attn.tile([128, 4, HDC], odt, tag="osb", bufs=3,
                                          name=f"o_sbq_{qt}")
                    for h in range(HPC):
                        rcp1 = attn.tile([128, 1], f32, tag="rcp", bufs=8,
                                         name=f"rcp_{c}_{h}")
                        nc.vector.reciprocal(rcp1[:], o_pss[h][:, 64:65])
                        eng = nc.scalar if h % 2 else nc.vector
                        if h % 2:
                            nc.scalar.activation(
                                o_sbq[:, cc, h * 64:(h + 1) * 64],
                                o_pss[h][:, 0:64], Act.Copy, scale=rcp1[:])
                        else:
                            nc.vector.tensor_scalar(
                                o_sbq[:, cc, h * 64:(h + 1) * 64],
                                o_pss[h][:, 0:64],
                                rcp1[:], None, op0=Alu.mult)
                    if True:
                        # per-chunk output DMA (overlaps the tail)
                        nc.sync.dma_start(out[c * 128:(c + 1) * 128, :],
                                          o_sbq[:, cc, :])


    nc.compile()
    return nc


def _get_nc():
    with _lock:
        if "nc" not in _cache:
            _cache["nc"] = _build_nc()
        return _cache["nc"]


def kernel(query, key, value, query_kernel, key_kernel, value_kernel):
    import ml_dtypes
    from concourse.bass_utils import run_bass_kernel_spmd

    nc = _get_nc()
    bf = ml_dtypes.bfloat16

    xT = {}
    for b in range(B):
        xT[("q", b)] = np.ascontiguousarray(query[b].T).astype(bf)
        xT[("k", b)] = np.ascontiguousarray(key[b].T).astype(bf)
        xT[("v", b)] = np.ascontiguousarray(value[b].T).astype(bf)

    in_maps = []
    for c in range(N_CORES):
        b, h0 = c // 4, 4 * (c % 4)
        in_maps.append({
            "xqT": xT[("q", b)],
            "xkT": xT[("k", b)],
            "xvT": xT[("v", b)],
            "wq": np.ascontiguousarray(
                query_kernel[:, h0:h0 + HPC, :].reshape(D, HDC)).astype(bf),
            "wk": np.ascontiguousarray(
                key_kernel[:, h0:h0 + HPC, :].reshape(D, HDC)).astype(bf),
            "wv": np.ascontiguousarray(
                value_kernel[:, h0:h0 + HPC, :].reshape(D, HDC)).astype(bf),
        })

    results = run_bass_kernel_spmd(nc, in_maps, core_ids=list(range(N_CORES)))

    # The reference ends with a FLAT reshape of [B*H, S, HD] -> (B, S, H*HD):
    # output rows [128h:128h+128] of batch b are head h's [S, HD] attention
    # output flat-reshaped to [128, H*HD].
    full = np.empty((B, S, H * HD), dtype=np.float32)
    for c in range(N_CORES):
        b, h0 = c // 4, 4 * (c % 4)
        av = results.results[c]["out"].astype(np.float32).reshape(S, HPC, HD)
        for hl in range(HPC):
            full[b, (h0 + hl) * 128:(h0 + hl + 1) * 128, :] = (
                av[:, hl, :].reshape(128, H * HD))
    return full
